# revision 6
# baseline (speedup 1.0000x reference)
"""Box-from-mask kernel for Trainium2 (8 NeuronCores, SPMD data-parallel).

Problem: masks [100, 800, 1280] f32 -> boxes [100, 2, 2] f32 where
box[n] = [[xmin, ymin], [xmax, ymax]] of {(y, x) : masks[n, y, x] > 0.5},
with empty-mask sentinels xmin=W, ymin=H, xmax=-1, ymax=-1.

Sharding: the flattened row axis (100*800 = 80000 rows of 1280 px) splits
into 8 contiguous shards of 10000 rows (= 25 half-mask "units" of 400 rows
each). Each core streams its shard once; the measured bottleneck is the
~358 GB/s per-core HBM read cap, so everything else must hide under the
stream and the post-stream drain must be short.

Device pipeline per core, per DMA chunk of S 128-row tiles (tile-major
layout: partition p of tile i holds row base + 128*i + p, one 5120 B
descriptor per row - the fastest measured HBM pattern):
  - per tile: one elementwise pass (DVE is_gt / ACT relu alternating)
    produces a 0/1-ish bf16 tile whose accum_out gives per-row "any";
  - PE matmul with a 25-column routing weight (a sliding one-hot window
    for single-unit tiles, custom windows for the 21 boundary tiles)
    accumulates per-column counts into each row's unit.
Column counts accumulate in two PSUM generations: the main one stops at
tile 71 so its (count>0) pass and 64 KB result DMA hide under the stream;
the tapering tail chunks (3,2,1 tiles - rows 9216..9983, units 23-24 only)
go to a tiny [2, W] generation resolved after the last byte. The final 16
rows (10000 = 78*128 + 16) are fetched as a [128, 160] tile (8 partitions
per row) and routed by an (x-phase -> PSUM row) weight into an [8, 160]
PSUM tile. The last full tile's elementwise pass is split into 640-px
halves on DVE || ACT to shorten the drain. Row/col "any" bitmaps
(~110 KB/core) ship to the host, which computes the min/max box
coordinates exactly in numpy.
"""

import sys

for _p in ("/opt/trn_rl_repo", "/opt/pypackages"):
    if _p not in sys.path:
        sys.path.append(_p)

import ml_dtypes
import numpy as np

import concourse.bass as bass
import concourse.tile as tile
from concourse import bacc, mybir
from concourse.bass_utils import run_bass_kernel_spmd

N, H, W = 100, 800, 1280
N_CORES = 8
THRESHOLD = 0.5

HU = 400  # rows per unit (half mask)
K = 25  # units per core
R = K * HU  # rows per core (10000)
MAIN_CHUNKS = [6] * 12  # 128-row tiles per stream DMA in the main phase
TAPER_CHUNKS = [3, 2, 1]  # tapering tail chunks (all inside units 23-24)
CHUNKS = MAIN_CHUNKS + TAPER_CHUNKS
NMAIN = sum(MAIN_CHUNKS)  # 72 tiles -> main PSUM generation
NSUB = sum(CHUNKS)  # 78 full 128-row tiles
BTAPER = 128 * NMAIN  # first taper row (9216); units 23..24 from here on
RUNT = R - 128 * NSUB  # 16 trailing rows, fetched as [128, RUNT*W/128]
RW = RUNT * W // 128  # 160 elems/partition, 8 partitions per row
PPR = 128 // RUNT  # runt partitions per row (8)
MAXS = max(CHUNKS)
HALF = W // 2  # split point of the last tile's elementwise pass

fp32 = mybir.dt.float32
fp16 = mybir.dt.float16
bf16 = mybir.dt.bfloat16
Op = mybir.AluOpType

_chunk_cols = [(0, 512), (512, 512), (1024, 256)]


def _tile_units(s):
    """(first_unit, n_units) covered by 128-row tile s."""
    u0 = (128 * s) // HU
    u1 = (128 * s + 127) // HU
    return u0, u1 - u0 + 1


_boundary = [s for s in range(NSUB) if _tile_units(s)[1] > 1]
# weight const layout: [oh bank 2K cols][custom windows 25 cols each][runt 8]
_OH = 2 * K
_cust_off = {s: _OH + 25 * i for i, s in enumerate(_boundary)}
WCOLS = _OH + 25 * len(_boundary) + 8


def _wslice(s):
    """(col0, width) of tile s's routing window in the weight const."""
    if s in _cust_off:
        return _cust_off[s], K if s < NMAIN else 2
    u0, _ = _tile_units(s)
    if s < NMAIN:
        return K - 1 - u0, K  # one-hot at local col u0
    return K - 1 - (u0 - (K - 2)), 2  # taper: units 23/24 -> cols 0/1


def build_program():
    """One-core Bass/Tile program; run SPMD on all 8 cores."""
    nc = bacc.Bacc(
        "TRN2", target_bir_lowering=False, debug=False, enable_asserts=False
    )
    masks = nc.dram_tensor("masks", [R, W], fp32, kind="ExternalInput").ap()
    wmat = nc.dram_tensor("wmat", [128, WCOLS], bf16, kind="ExternalInput").ap()
    rowany_out = nc.dram_tensor(
        "rowany_out", [128, NSUB + 2], fp32, kind="ExternalOutput"
    ).ap()
    colany_out = nc.dram_tensor("colany_out", [K, W], fp16, kind="ExternalOutput").ap()
    colb16_out = nc.dram_tensor("colb16_out", [2, 768], fp16, kind="ExternalOutput").ap()
    colb32_out = nc.dram_tensor("colb32_out", [2, 512], fp32, kind="ExternalOutput").ap()
    runt_out = nc.dram_tensor("runt_out", [8, RW], fp16, kind="ExternalOutput").ap()

    with tile.TileContext(nc) as tc:
        with (
            tc.tile_pool(name="raw", bufs=4) as rawp,
            tc.tile_pool(name="bin", bufs=12) as binp,
            tc.tile_pool(name="consts", bufs=1) as constp,
            tc.tile_pool(name="psum", bufs=1, space="PSUM") as psump,
        ):
            # consts ride gpsimd SWDGE queues so SP's HWDGE queues start
            # streaming mask chunks immediately
            wmat_t = constp.tile([128, WCOLS], bf16)
            nc.gpsimd.dma_start(wmat_t[:], wmat)
            rowany = constp.tile([128, NSUB + 2], fp32)
            nc.gpsimd.memset(rowany[:], 0.0)
            negh = constp.tile([128, 1], fp32)
            nc.gpsimd.memset(negh[:], -THRESHOLD)
            cola_sb = constp.tile([K, W], fp16)
            colb16_sb = constp.tile([2, 768], fp16)
            colb32_sb = constp.tile([2, 512], fp32)
            runt_sb = constp.tile([8, RW], fp16)
            cca = [
                psump.tile([K, cw], fp32, name=f"cca{ci}", tag=f"cca{ci}")
                for ci, (_, cw) in enumerate(_chunk_cols)
            ]
            ccb = [
                psump.tile([2, cw], fp32, name=f"ccb{ci}", tag=f"ccb{ci}")
                for ci, (_, cw) in enumerate(_chunk_cols)
            ]
            ccr = psump.tile([8, RW], fp32, name="ccr", tag="ccr")

            eng_flip = [0]

            def binarize(out_b, rv, acc, force=None):
                """One elementwise pass: binary tile for PE + row-any accum."""
                eng = force if force else ("dve" if eng_flip[0] % 2 == 0 else "act")
                if eng == "dve":
                    nc.vector.tensor_scalar(
                        out=out_b,
                        in0=rv,
                        scalar1=THRESHOLD,
                        scalar2=None,
                        op0=Op.is_gt,
                        op1=Op.max,
                        accum_out=acc,
                    )
                else:
                    nc.scalar.activation(
                        out=out_b,
                        in_=rv,
                        func=mybir.ActivationFunctionType.Relu,
                        bias=negh[:, :],
                        scale=1.0,
                        accum_out=acc,
                    )
                if force is None:
                    eng_flip[0] += 1

            def routed_matmuls(s, b):
                cc = cca if s < NMAIN else ccb
                w0, wn = _wslice(s)
                for ci, (c0, cw) in enumerate(_chunk_cols):
                    nc.tensor.matmul(
                        cc[ci][:, :],
                        wmat_t[:, w0 : w0 + wn],
                        b[:, c0 : c0 + cw],
                        start=(s == 0 or s == NMAIN),
                        stop=(s == NMAIN - 1 or s == NSUB - 1),
                    )

            s = 0
            base = 0
            for S in CHUNKS:
                raw = rawp.tile([128, MAXS * W], fp32, tag="raw")
                nc.sync.dma_start(
                    raw[:, : S * W],
                    masks[base : base + 128 * S, :].rearrange(
                        "(a p) x -> p a x", p=128
                    ),
                )
                for i in range(S):
                    b = binp.tile([128, W], bf16, tag="b")
                    rv = raw[:, i * W : (i + 1) * W]
                    if s == NSUB - 1:
                        # final tile: halves on DVE || ACT to shorten drain
                        binarize(
                            b[:, :HALF], rv[:, :HALF],
                            rowany[:, s : s + 1], force="dve",
                        )
                        binarize(
                            b[:, HALF:], rv[:, HALF:],
                            rowany[:, NSUB + 1 : NSUB + 2], force="act",
                        )
                    else:
                        binarize(b[:], rv, rowany[:, s : s + 1])
                    routed_matmuls(s, b)
                    s += 1
                base += 128 * S
                if s == NMAIN:
                    # main generation closed: (count>0) + result DMA hide
                    # under the taper stream (gpsimd cannot read PSUM, so
                    # the compare runs on DVE; the stream has compute slack)
                    for ci, (c0, cw) in enumerate(_chunk_cols):
                        nc.vector.tensor_scalar(
                            out=cola_sb[:, c0 : c0 + cw],
                            in0=cca[ci][:, :],
                            scalar1=0.0,
                            scalar2=None,
                            op0=Op.is_gt,
                        )
                    nc.gpsimd.dma_start(colany_out, cola_sb[:])

            # trailing 16 rows as [128, 160]: partition p = row base + p//8,
            # x in [160*(p%8), 160*(p%8)+160)
            raw = rawp.tile([128, MAXS * W], fp32, tag="raw")
            nc.sync.dma_start(
                raw[:, :RW],
                masks[base:R, :].rearrange("y (u a) -> (y u) a", u=PPR),
            )
            br = binp.tile([128, W], bf16, tag="b")
            binarize(br[:, :RW], raw[:, :RW], rowany[:, NSUB : NSUB + 1], force="dve")
            nc.tensor.matmul(
                ccr[:, :],
                wmat_t[:, WCOLS - 8 : WCOLS],
                br[:, :RW],
                start=True,
                stop=True,
            )
            nc.vector.tensor_scalar(
                out=runt_sb[:], in0=ccr[:, :], scalar1=0.0, scalar2=None, op0=Op.is_gt
            )
            nc.gpsimd.dma_start(runt_out, runt_sb[:])
            nc.gpsimd.dma_start(rowany_out, rowany[:])

            # taper-generation (count>0): chunks 0/2 on DVE, chunk 1 on ACT
            # (relu keeps positives positive; f32 out so tiny relu-sums
            # cannot round to zero)
            nc.vector.tensor_scalar(
                out=colb16_sb[:, 0:512],
                in0=ccb[0][:, :],
                scalar1=0.0,
                scalar2=None,
                op0=Op.is_gt,
            )
            nc.scalar.activation(
                out=colb32_sb[:],
                in_=ccb[1][:, :],
                func=mybir.ActivationFunctionType.Relu,
                bias=0.0,
                scale=1.0,
            )
            nc.vector.tensor_scalar(
                out=colb16_sb[:, 512:768],
                in0=ccb[2][:, :],
                scalar1=0.0,
                scalar2=None,
                op0=Op.is_gt,
            )
            nc.scalar.dma_start(colb32_out, colb32_sb[:])
            nc.scalar.dma_start(colb16_out, colb16_sb[:])

    nc.compile()
    return nc


def make_wmat():
    """Routing weights: sliding one-hot bank + boundary windows + runt."""
    wmat = np.zeros((128, WCOLS), ml_dtypes.bfloat16)
    p = np.arange(128)
    wmat[:, K - 1] = 1  # one-hot bank: col K-1
    for s, off in _cust_off.items():
        units = (128 * s + p) // HU
        wmat[p, off + units - (0 if s < NMAIN else K - 2)] = 1
    wmat[p, WCOLS - 8 + p % PPR] = 1
    return wmat


_cache = {}


def _get_program():
    if "nc" not in _cache:
        _cache["nc"] = build_program()
        _cache["wmat"] = make_wmat()
    return _cache["nc"], _cache["wmat"]


def make_in_maps(masks):
    masks = np.ascontiguousarray(np.asarray(masks, dtype=np.float32))
    _, wmat = _get_program()
    rows = masks.reshape(N_CORES, R, W)
    return [{"masks": rows[c], "wmat": wmat} for c in range(N_CORES)]


def postprocess(results):
    """Per-core any-bitmaps -> boxes [N, 2, 2] f32 (exact integer math)."""
    nu = N_CORES * K  # 200 units (half masks)
    u_ymin = np.full(nu, float(H))
    u_ymax = np.full(nu, -1.0)
    u_xmin = np.full(nu, float(W))
    u_xmax = np.full(nu, -1.0)
    ys = np.arange(HU)
    xs = np.arange(W)
    for c, r in enumerate(results):
        ra = np.asarray(r["rowany_out"], np.float32)
        rows_any = np.empty(R, bool)
        s = 0
        base = 0
        for S in CHUNKS:
            blk = ra[:, s : s + S] > 0  # [128, S]; row = base + 128*i + p
            rows_any[base : base + 128 * S] = blk.T.reshape(-1)
            s += S
            base += 128 * S
        rows_any[base - 128 : base] |= ra[:, NSUB + 1] > 0  # last tile 2nd half
        rows_any[base:] = (ra[:, NSUB] > 0).reshape(RUNT, PPR).any(1)

        ca = np.asarray(r["colany_out"], np.float32) > 0  # [K, W] units 0-24
        b16 = np.asarray(r["colb16_out"], np.float32) > 0  # [2, 768]
        b32 = np.asarray(r["colb32_out"], np.float32) > 0  # [2, 512]
        cb = np.concatenate([b16[:, 0:512], b32, b16[:, 512:768]], axis=1)
        ca[K - 2] |= cb[0]
        ca[K - 1] |= cb[1]
        ca[K - 1] |= (np.asarray(r["runt_out"], np.float32) > 0).reshape(W)

        A = rows_any.reshape(K, HU)
        g = c * K + np.arange(K)
        off = (g % 2) * HU  # row offset of this unit within its mask
        has = A.any(1)
        u_ymin[g] = np.where(has, off + np.where(A, ys, H).min(1), H)
        u_ymax[g] = np.where(has, off + np.where(A, ys, -1).max(1), -1)
        hasx = ca.any(1)
        u_xmin[g] = np.where(hasx, np.where(ca, xs, W).min(1), W)
        u_xmax[g] = np.where(hasx, np.where(ca, xs, -1).max(1), -1)

    boxes = np.empty((N, 2, 2), np.float32)
    boxes[:, 0, 0] = u_xmin.reshape(N, 2).min(1)
    boxes[:, 0, 1] = u_ymin.reshape(N, 2).min(1)
    boxes[:, 1, 0] = u_xmax.reshape(N, 2).max(1)
    boxes[:, 1, 1] = u_ymax.reshape(N, 2).max(1)
    return boxes


def kernel(masks):
    nc, _ = _get_program()
    in_maps = make_in_maps(masks)
    res = run_bass_kernel_spmd(nc, in_maps, core_ids=list(range(N_CORES)))
    return postprocess(res.results)
